# revision 5
# baseline (speedup 1.0000x reference)
"""MoE layer (8 experts, top-2) on 8 Trainium2 NeuronCores.

Strategy: expert parallelism. The router (x @ gate_w.T -> top-2 -> softmax)
is computed on host in fp32 (0.03% of total FLOPs); tokens are then
sharded BY EXPERT: core e receives the tokens routed to expert e (padded
to a fixed capacity C) plus expert e's weights, pre-packed into
DMA-friendly tiled layouts. Each core computes the dense expert MLP
    y = gelu(x @ w1[e].T + b1[e]) @ w2[e].T
in bf16 (fp32 PSUM accumulation). The combine (scatter-add weighted by the
top-2 softmax coefficients, + b2 since the coefficients sum to 1) happens
on host as the unshard step.

Device kernel layout (per core, SPMD identical program):
  xT  [D_IN, C]  bf16   tokens, transposed (partition dim = contraction)
  w1p [64, 128, 2048]   w1 tiles: w1p[h0, p, kt*128+h] = w1[e][h0*128+h, kt*128+p]
  w2p [8, 4, 128, 4096] w2 tiles: w2p[b, dc, p, i*512+d] = w2[e][dc*512+d, b*1024+i*128+p]
  b1c [128, 64]         b1 per hid-tile column
  y   [C, D_OUT] f32    expert output (excl. b2, excl. routing coef)

Inner loop: token groups of TG=1152; hid blocked by 1024 (h stays in SBUF
as bf16); y accumulated in SBUF fp32 via DVE adds across hid blocks.
"""

import numpy as np
import ml_dtypes

TOP_K = 2
NUM_EXPERTS = 8
D_IN, D_HID, D_OUT = 2048, 8192, 2048

P = 128
TG = 1152          # tokens per group (SBUF-resident)
TOKW = 384         # matmul-1 moving width; TG = 3 * TOKW
DOUTW = 512        # matmul-2 moving width; D_OUT = 4 * DOUTW
KT = D_IN // P     # 16 contraction tiles
NBLK = 8           # hid blocks of 1024
HPB = 8            # hid 128-tiles per block
NT = TG // P       # 9 token 128-tiles per group
NTW = TG // TOKW   # 3
NDC = D_OUT // DOUTW  # 4

_BF16 = ml_dtypes.bfloat16

_nc_cache: dict[int, object] = {}

LAST_EXEC_TIME_NS = None
LAST_RESULTS = None


def _groups_for(max_c: int) -> tuple[int, ...]:
    """Token-group sizes (<= TG) covering exactly max_c tokens."""
    c = max(max_c, 1)
    groups = []
    while c > TG:
        groups.append(TG)
        c -= TG
    groups.append(c)
    return tuple(groups)


def _widths_for(tg: int) -> list[int]:
    """Split a group into balanced matmul-1 moving widths (<= 512)."""
    if tg % 384 == 0 and tg % 512 != 0:
        return [384] * (tg // 384)
    n = -(-tg // 512)
    base, rem = divmod(tg, n)
    return [base + 1] * rem + [base] * (n - rem)


def _build_bass(groups: tuple[int, ...]):
    from concourse import bacc
    import concourse.mybir as mybir
    import concourse.tile as tile

    bf16 = mybir.dt.bfloat16
    f32 = mybir.dt.float32
    C = sum(groups)
    tgmax = max(groups)

    nc = bacc.Bacc("TRN2", target_bir_lowering=False, debug=False,
                   num_devices=NUM_EXPERTS)
    xT = nc.declare_dram_parameter("xT", [D_IN, C], bf16, isOutput=False)
    w1p = nc.declare_dram_parameter("w1p", [D_HID // P, P, D_IN], bf16,
                                    isOutput=False)
    w2p = nc.declare_dram_parameter("w2p", [NBLK, NDC, P, HPB * DOUTW], bf16,
                                    isOutput=False)
    b1c = nc.declare_dram_parameter("b1c", [P, D_HID // P], f32, isOutput=False)
    y = nc.declare_dram_parameter("y", [C, D_OUT], f32, isOutput=True)

    gelu = mybir.ActivationFunctionType.Gelu

    with tile.TileContext(nc) as tc:
        with (
            tc.tile_pool(name="consts", bufs=1) as cpool,
            tc.tile_pool(name="xpool", bufs=1) as xpool,
            tc.tile_pool(name="ypool", bufs=1) as ypool,
            tc.tile_pool(name="hpool", bufs=2) as hpool,
            tc.tile_pool(name="w1pool", bufs=3) as w1pool,
            tc.tile_pool(name="w2pool", bufs=3) as w2pool,
            tc.tile_pool(name="phpool", bufs=4, space="PSUM") as phpool,
            tc.tile_pool(name="pypool", bufs=4, space="PSUM") as pypool,
        ):
            b1t = cpool.tile([P, D_HID // P], f32)
            nc.sync.dma_start(b1t[:], b1c[:])
            # prefetch the first w1 tile so the first matmul group is not
            # stuck behind the 16 x-tile DMAs
            w1_pre = w1pool.tile([P, D_IN], bf16, tag="w1")
            nc.sync.dma_start(w1_pre[:], w1p[0])

            g0 = 0
            for g, tg in enumerate(groups):
                widths = _widths_for(tg)
                nt = -(-tg // P)
                xs = [xpool.tile([P, tgmax], bf16, tag=f"x{kt}",
                                 name=f"xs{kt}")
                      for kt in range(KT)]
                for kt in range(KT):
                    nc.sync.dma_start(
                        xs[kt][:, :tg], xT[kt * P:(kt + 1) * P, g0:g0 + tg])
                ys = [ypool.tile([P, D_OUT], f32, tag=f"y{t}", name=f"ys{t}")
                      for t in range(nt)]
                for b in range(NBLK):
                    hs = [hpool.tile([P, tgmax], bf16, tag=f"h{i}",
                                     name=f"hs{i}")
                          for i in range(HPB)]
                    # ---- matmul 1: h[hid, tok] = w1 @ x, gelu ----
                    for hb in range(HPB):
                        hid0 = b * HPB + hb
                        if g == 0 and b == 0 and hb == 0:
                            w1t = w1_pre
                        else:
                            w1t = w1pool.tile([P, D_IN], bf16, tag="w1")
                            nc.sync.dma_start(w1t[:], w1p[hid0])
                        tw0 = 0
                        for tw in widths:
                            ph = phpool.tile([P, 512], mybir.dt.float32,
                                             tag="ph")
                            for kt in range(KT):
                                nc.tensor.matmul(
                                    ph[:, :tw],
                                    w1t[:, kt * P:(kt + 1) * P],
                                    xs[kt][:, tw0:tw0 + tw],
                                    start=(kt == 0), stop=(kt == KT - 1))
                            nc.scalar.activation(
                                hs[hb][:, tw0:tw0 + tw], ph[:, :tw],
                                gelu, bias=b1t[:, hid0:hid0 + 1])
                            tw0 += tw
                    # ---- matmul 2: y[tok, dout] += h_blk @ w2_blk ----
                    for dc in range(NDC):
                        w2t = w2pool.tile([P, HPB * DOUTW], bf16, tag="w2")
                        nc.sync.dma_start(w2t[:], w2p[b, dc])
                        for t in range(nt):
                            pt = min(P, tg - t * P)
                            py = pypool.tile([P, DOUTW], mybir.dt.float32,
                                             tag="py")
                            for i in range(HPB):
                                nc.tensor.matmul(
                                    py[:pt, :],
                                    hs[i][:, t * P:t * P + pt],
                                    w2t[:, i * DOUTW:(i + 1) * DOUTW],
                                    start=(i == 0), stop=(i == HPB - 1))
                            dst = ys[t][:pt, dc * DOUTW:(dc + 1) * DOUTW]
                            if b == 0:
                                nc.vector.tensor_copy(dst, py[:pt, :])
                            else:
                                nc.vector.tensor_add(dst, dst, py[:pt, :])
                                if b == NBLK - 1:
                                    nc.sync.dma_start(
                                        y[g0 + t * P:g0 + t * P + pt,
                                          dc * DOUTW:(dc + 1) * DOUTW],
                                        dst)
                g0 += tg
    nc.compile()
    return nc


def kernel(x, gate_w, w1, b1, w2, b2):
    global LAST_EXEC_TIME_NS, LAST_RESULTS
    x = np.asarray(x, dtype=np.float32)
    gate_w = np.asarray(gate_w, dtype=np.float32)
    w1 = np.asarray(w1, dtype=np.float32)
    b1 = np.asarray(b1, dtype=np.float32)
    w2 = np.asarray(w2, dtype=np.float32)
    b2 = np.asarray(b2, dtype=np.float32)
    B = x.shape[0]

    # ---- host router (fp32, matches jax.lax.top_k tie-breaking) ----
    logits = x @ gate_w.T                                     # [B, E]
    order = np.argsort(-logits, axis=1, kind="stable")[:, :TOP_K]
    top_v = np.take_along_axis(logits, order, axis=1)
    mx = top_v.max(axis=1, keepdims=True)
    ex = np.exp(top_v - mx)
    coefs = ex / ex.sum(axis=1, keepdims=True)                # [B, 2]

    toks, cfs = [], []
    for e in range(NUM_EXPERTS):
        mask = order == e                                     # [B, 2]
        tok = np.nonzero(mask.any(axis=1))[0]
        first = mask[tok, 0]
        cf = np.where(first, coefs[tok, 0], coefs[tok, 1]).astype(np.float32)
        toks.append(tok)
        cfs.append(cf)

    max_c = max(len(t) for t in toks)
    groups = _groups_for(max_c)
    C = sum(groups)

    # ---- per-core inputs: tokens + packed weights of the owned expert ----
    in_maps = []
    for e in range(NUM_EXPERTS):
        tok = toks[e]
        xg = np.zeros((C, D_IN), np.float32)
        xg[:len(tok)] = x[tok]
        xT = xg.T.astype(_BF16)                               # [D_IN, C]

        w1e = w1[e].astype(_BF16)                             # [HID, D_IN]
        w1p = (w1e.reshape(D_HID // P, P, KT, P)
               .transpose(0, 3, 2, 1)
               .reshape(D_HID // P, P, D_IN))
        w1p = np.ascontiguousarray(w1p)

        w2e = w2[e].astype(_BF16)                             # [D_OUT, HID]
        w2p = (w2e.reshape(NDC, DOUTW, NBLK, HPB, P)
               .transpose(2, 0, 4, 3, 1)
               .reshape(NBLK, NDC, P, HPB * DOUTW))
        w2p = np.ascontiguousarray(w2p)

        b1c = np.ascontiguousarray(b1[e].reshape(D_HID // P, P).T)

        in_maps.append({"xT": xT, "w1p": w1p, "w2p": w2p, "b1c": b1c})

    nc = _nc_cache.get(groups)
    if nc is None:
        nc = _build_bass(groups)
        _nc_cache[groups] = nc

    from concourse.bass_utils import run_bass_kernel_spmd
    res = run_bass_kernel_spmd(nc, in_maps, core_ids=list(range(NUM_EXPERTS)))
    LAST_EXEC_TIME_NS = res.exec_time_ns
    LAST_RESULTS = res

    # ---- combine (unshard): weighted scatter-add; b2[e] folded in here ----
    out = np.zeros((B, D_OUT), np.float32)
    for e in range(NUM_EXPERTS):
        tok = toks[e]
        y_e = np.asarray(res.results[e]["y"])[:len(tok)]
        out[tok] += (y_e + b2[e][None, :]) * cfs[e][:, None]
    return out


# revision 6
# speedup vs baseline: 1.0561x; 1.0561x over previous
"""MoE layer (8 experts, top-2) on 8 Trainium2 NeuronCores.

Strategy: expert parallelism. The router (x @ gate_w.T -> top-2 -> softmax)
is computed on host in fp32 (0.03% of total FLOPs); tokens are then
sharded BY EXPERT: core e receives the tokens routed to expert e (padded
to a fixed capacity C) plus expert e's weights, pre-packed into
DMA-friendly tiled layouts. Each core computes the dense expert MLP
    y = gelu(x @ w1[e].T + b1[e]) @ w2[e].T
in bf16 (fp32 PSUM accumulation). The combine (scatter-add weighted by the
top-2 softmax coefficients, + b2 since the coefficients sum to 1) happens
on host as the unshard step.

Device kernel layout (per core, SPMD identical program):
  xT  [D_IN, C]  bf16   tokens, transposed (partition dim = contraction)
  w1p [64, 128, 2048]   w1 tiles: w1p[h0, p, kt*128+h] = w1[e][h0*128+h, kt*128+p]
  w2p [8, 4, 128, 4096] w2 tiles: w2p[b, dc, p, i*512+d] = w2[e][dc*512+d, b*1024+i*128+p]
  b1c [128, 64]         b1 per hid-tile column
  y   [C, D_OUT] f32    expert output (excl. b2, excl. routing coef)

Inner loop: token groups of TG=1152; hid blocked by 1024 (h stays in SBUF
as bf16); y accumulated in SBUF fp32 via DVE adds across hid blocks.
"""

import numpy as np
import ml_dtypes

TOP_K = 2
NUM_EXPERTS = 8
D_IN, D_HID, D_OUT = 2048, 8192, 2048

P = 128
TG = 1152          # tokens per group (SBUF-resident)
TOKW = 384         # matmul-1 moving width; TG = 3 * TOKW
DOUTW = 512        # matmul-2 moving width; D_OUT = 4 * DOUTW
KT = D_IN // P     # 16 contraction tiles
NBLK = 8           # hid blocks of 1024
HPB = 8            # hid 128-tiles per block
NT = TG // P       # 9 token 128-tiles per group
NTW = TG // TOKW   # 3
NDC = D_OUT // DOUTW  # 4

_BF16 = ml_dtypes.bfloat16

_nc_cache: dict[int, object] = {}

LAST_EXEC_TIME_NS = None
LAST_RESULTS = None


def _groups_for(max_c: int) -> tuple[int, ...]:
    """Token-group sizes (multiples of 128, <= TG) covering max_c."""
    c = max(P, -(-max_c // P) * P)
    groups = []
    while c > TG:
        groups.append(TG)
        c -= TG
    groups.append(c)
    return tuple(groups)


def _widths_for(tg: int) -> list[int]:
    """Split a group into 128-aligned matmul-1 moving widths (<= 512)."""
    if tg % 384 == 0 and tg % 512 != 0:
        return [384] * (tg // 384)
    ws = [512] * (tg // 512)
    if tg % 512:
        ws.append(tg % 512)
    return ws


def _build_bass(groups: tuple[int, ...]):
    from concourse import bacc
    import concourse.mybir as mybir
    import concourse.tile as tile

    bf16 = mybir.dt.bfloat16
    f32 = mybir.dt.float32
    C = sum(groups)
    tgmax = max(groups)

    nc = bacc.Bacc("TRN2", target_bir_lowering=False, debug=False,
                   num_devices=NUM_EXPERTS)
    xT = nc.declare_dram_parameter("xT", [D_IN, C], bf16, isOutput=False)
    w1p = nc.declare_dram_parameter("w1p", [D_HID // P, P, D_IN], bf16,
                                    isOutput=False)
    w2p = nc.declare_dram_parameter("w2p", [NBLK, NDC, P, HPB * DOUTW], bf16,
                                    isOutput=False)
    b1c = nc.declare_dram_parameter("b1c", [P, D_HID // P], f32, isOutput=False)
    y = nc.declare_dram_parameter("y", [C, D_OUT], f32, isOutput=True)

    gelu = mybir.ActivationFunctionType.Gelu

    with tile.TileContext(nc) as tc:
        with (
            tc.tile_pool(name="consts", bufs=1) as cpool,
            tc.tile_pool(name="xpool", bufs=1) as xpool,
            tc.tile_pool(name="ypool", bufs=1) as ypool,
            tc.tile_pool(name="hpool", bufs=2) as hpool,
            tc.tile_pool(name="w1pool", bufs=3) as w1pool,
            tc.tile_pool(name="w2pool", bufs=3) as w2pool,
            tc.tile_pool(name="phpool", bufs=4, space="PSUM") as phpool,
            tc.tile_pool(name="pypool", bufs=4, space="PSUM") as pypool,
        ):
            b1t = cpool.tile([P, D_HID // P], f32)
            nc.sync.dma_start(b1t[:], b1c[:])
            # prefetch the first w1 tile so the first matmul group is not
            # stuck behind the 16 x-tile DMAs
            w1_pre = w1pool.tile([P, D_IN], bf16, tag="w1")
            nc.sync.dma_start(w1_pre[:], w1p[0])

            g0 = 0
            for g, tg in enumerate(groups):
                widths = _widths_for(tg)
                nt = -(-tg // P)
                xs = [xpool.tile([P, tgmax], bf16, tag=f"x{kt}",
                                 name=f"xs{kt}")
                      for kt in range(KT)]
                for kt in range(KT):
                    nc.sync.dma_start(
                        xs[kt][:, :tg], xT[kt * P:(kt + 1) * P, g0:g0 + tg])
                ys = [ypool.tile([P, D_OUT], f32, tag=f"y{t}", name=f"ys{t}")
                      for t in range(nt)]
                for b in range(NBLK):
                    hs = [hpool.tile([P, tgmax], bf16, tag=f"h{i}",
                                     name=f"hs{i}")
                          for i in range(HPB)]
                    # ---- matmul 1: h[hid, tok] = w1 @ x, gelu ----
                    for hb in range(HPB):
                        hid0 = b * HPB + hb
                        if g == 0 and b == 0 and hb == 0:
                            w1t = w1_pre
                        else:
                            w1t = w1pool.tile([P, D_IN], bf16, tag="w1")
                            nc.sync.dma_start(w1t[:], w1p[hid0])
                        tw0 = 0
                        for tw in widths:
                            ph = phpool.tile([P, 512], mybir.dt.float32,
                                             tag="ph")
                            for kt in range(KT):
                                nc.tensor.matmul(
                                    ph[:, :tw],
                                    w1t[:, kt * P:(kt + 1) * P],
                                    xs[kt][:, tw0:tw0 + tw],
                                    start=(kt == 0), stop=(kt == KT - 1))
                            nc.scalar.activation(
                                hs[hb][:, tw0:tw0 + tw], ph[:, :tw],
                                gelu, bias=b1t[:, hid0:hid0 + 1])
                            tw0 += tw
                    # ---- matmul 2: y[tok, dout] += h_blk @ w2_blk ----
                    for dc in range(NDC):
                        w2t = w2pool.tile([P, HPB * DOUTW], bf16, tag="w2")
                        nc.sync.dma_start(w2t[:], w2p[b, dc])
                        for t in range(nt):
                            pt = min(P, tg - t * P)
                            py = pypool.tile([P, DOUTW], mybir.dt.float32,
                                             tag="py")
                            for i in range(HPB):
                                nc.tensor.matmul(
                                    py[:pt, :],
                                    hs[i][:, t * P:t * P + pt],
                                    w2t[:, i * DOUTW:(i + 1) * DOUTW],
                                    start=(i == 0), stop=(i == HPB - 1))
                            dst = ys[t][:pt, dc * DOUTW:(dc + 1) * DOUTW]
                            if b == 0:
                                nc.vector.tensor_copy(dst, py[:pt, :])
                            else:
                                nc.vector.tensor_add(dst, dst, py[:pt, :])
                                if b == NBLK - 1:
                                    nc.sync.dma_start(
                                        y[g0 + t * P:g0 + t * P + pt,
                                          dc * DOUTW:(dc + 1) * DOUTW],
                                        dst)
                g0 += tg
    nc.compile()
    return nc


def kernel(x, gate_w, w1, b1, w2, b2):
    global LAST_EXEC_TIME_NS, LAST_RESULTS
    x = np.asarray(x, dtype=np.float32)
    gate_w = np.asarray(gate_w, dtype=np.float32)
    w1 = np.asarray(w1, dtype=np.float32)
    b1 = np.asarray(b1, dtype=np.float32)
    w2 = np.asarray(w2, dtype=np.float32)
    b2 = np.asarray(b2, dtype=np.float32)
    B = x.shape[0]

    # ---- host router (fp32, matches jax.lax.top_k tie-breaking) ----
    logits = x @ gate_w.T                                     # [B, E]
    order = np.argsort(-logits, axis=1, kind="stable")[:, :TOP_K]
    top_v = np.take_along_axis(logits, order, axis=1)
    mx = top_v.max(axis=1, keepdims=True)
    ex = np.exp(top_v - mx)
    coefs = ex / ex.sum(axis=1, keepdims=True)                # [B, 2]

    toks, cfs = [], []
    for e in range(NUM_EXPERTS):
        mask = order == e                                     # [B, 2]
        tok = np.nonzero(mask.any(axis=1))[0]
        first = mask[tok, 0]
        cf = np.where(first, coefs[tok, 0], coefs[tok, 1]).astype(np.float32)
        toks.append(tok)
        cfs.append(cf)

    max_c = max(len(t) for t in toks)
    groups = _groups_for(max_c)
    C = sum(groups)

    # ---- per-core inputs: tokens + packed weights of the owned expert ----
    in_maps = []
    for e in range(NUM_EXPERTS):
        tok = toks[e]
        xg = np.zeros((C, D_IN), np.float32)
        xg[:len(tok)] = x[tok]
        xT = xg.T.astype(_BF16)                               # [D_IN, C]

        w1e = w1[e].astype(_BF16)                             # [HID, D_IN]
        w1p = (w1e.reshape(D_HID // P, P, KT, P)
               .transpose(0, 3, 2, 1)
               .reshape(D_HID // P, P, D_IN))
        w1p = np.ascontiguousarray(w1p)

        w2e = w2[e].astype(_BF16)                             # [D_OUT, HID]
        w2p = (w2e.reshape(NDC, DOUTW, NBLK, HPB, P)
               .transpose(2, 0, 4, 3, 1)
               .reshape(NBLK, NDC, P, HPB * DOUTW))
        w2p = np.ascontiguousarray(w2p)

        b1c = np.ascontiguousarray(b1[e].reshape(D_HID // P, P).T)

        in_maps.append({"xT": xT, "w1p": w1p, "w2p": w2p, "b1c": b1c})

    nc = _nc_cache.get(groups)
    if nc is None:
        nc = _build_bass(groups)
        _nc_cache[groups] = nc

    from concourse.bass_utils import run_bass_kernel_spmd
    res = run_bass_kernel_spmd(nc, in_maps, core_ids=list(range(NUM_EXPERTS)))
    LAST_EXEC_TIME_NS = res.exec_time_ns
    LAST_RESULTS = res

    # ---- combine (unshard): weighted scatter-add; b2[e] folded in here ----
    out = np.zeros((B, D_OUT), np.float32)
    for e in range(NUM_EXPERTS):
        tok = toks[e]
        y_e = np.asarray(res.results[e]["y"])[:len(tok)]
        out[tok] += (y_e + b2[e][None, :]) * cfs[e][:, None]
    return out


# revision 7
# speedup vs baseline: 1.2014x; 1.1375x over previous
"""MoE layer (8 experts, top-2) on 8 Trainium2 NeuronCores.

Strategy: expert parallelism. The router (x @ gate_w.T -> top-2 -> softmax)
is computed on host in fp32 (0.03% of total FLOPs); tokens are then
sharded BY EXPERT: core e receives the tokens routed to expert e (padded
to a fixed capacity C) plus expert e's weights, pre-packed into
DMA-friendly tiled layouts. Each core computes the dense expert MLP
    y = gelu(x @ w1[e].T + b1[e]) @ w2[e].T
in bf16 (fp32 PSUM accumulation). The combine (scatter-add weighted by the
top-2 softmax coefficients, + b2 since the coefficients sum to 1) happens
on host as the unshard step.

Device kernel layout (per core, SPMD identical program):
  xT  [D_IN, C]  bf16   tokens, transposed (partition dim = contraction)
  w1p [64, 128, 2048]   w1 tiles: w1p[h0, p, kt*128+h] = w1[e][h0*128+h, kt*128+p]
  w2p [8, 4, 128, 4096] w2 tiles: w2p[b, dc, p, i*512+d] = w2[e][dc*512+d, b*1024+i*128+p]
  b1c [128, 64]         b1 per hid-tile column
  y   [C, D_OUT] f32    expert output (excl. b2, excl. routing coef)

Inner loop: token groups of TG=1152; hid blocked by 1024 (h stays in SBUF
as bf16); y accumulated in SBUF fp32 via DVE adds across hid blocks.
"""

import numpy as np
import ml_dtypes

TOP_K = 2
NUM_EXPERTS = 8
D_IN, D_HID, D_OUT = 2048, 8192, 2048

P = 128
TG = 1152          # tokens per group (SBUF-resident)
TOKW = 384         # matmul-1 moving width; TG = 3 * TOKW
DOUTW = 512        # matmul-2 moving width; D_OUT = 4 * DOUTW
KT = D_IN // P     # 16 contraction tiles
NBLK = 8           # hid blocks of 1024
HPB = 8            # hid 128-tiles per block
NT = TG // P       # 9 token 128-tiles per group
NTW = TG // TOKW   # 3
NDC = D_OUT // DOUTW  # 4

_BF16 = ml_dtypes.bfloat16

_nc_cache: dict[int, object] = {}

LAST_EXEC_TIME_NS = None
LAST_RESULTS = None


def _groups_for(max_c: int) -> tuple[int, ...]:
    """Token-group sizes (<= TG) covering exactly max_c tokens."""
    c = max(max_c, 1)
    groups = []
    while c > TG:
        groups.append(TG)
        c -= TG
    groups.append(c)
    return tuple(groups)


def _widths_for(tg: int) -> list[int]:
    """Split a group into matmul-1 moving widths (<= 512), each starting at
    a 128-aligned token offset (only the last may be a non-multiple)."""
    if tg % 384 == 0 and tg % 512 != 0:
        return [384] * (tg // 384)
    ws = [512] * (tg // 512)
    if tg % 512:
        ws.append(tg % 512)
    return ws


def _build_bass(groups: tuple[int, ...]):
    from concourse import bacc
    import concourse.mybir as mybir
    import concourse.tile as tile

    bf16 = mybir.dt.bfloat16
    f32 = mybir.dt.float32
    C = sum(groups)
    tgmax = max(groups)

    nc = bacc.Bacc("TRN2", target_bir_lowering=False, debug=False,
                   num_devices=NUM_EXPERTS)
    xT = nc.declare_dram_parameter("xT", [D_IN, C], bf16, isOutput=False)
    w1p = nc.declare_dram_parameter("w1p", [D_HID // P, P, D_IN], bf16,
                                    isOutput=False)
    w2p = nc.declare_dram_parameter("w2p", [NBLK, NDC, P, HPB * DOUTW], bf16,
                                    isOutput=False)
    b1c = nc.declare_dram_parameter("b1c", [P, D_HID // P], f32, isOutput=False)
    y = nc.declare_dram_parameter("y", [C, D_OUT], f32, isOutput=True)

    gelu = mybir.ActivationFunctionType.Gelu

    with tile.TileContext(nc) as tc:
        with (
            tc.tile_pool(name="consts", bufs=1) as cpool,
            tc.tile_pool(name="xpool", bufs=1) as xpool,
            tc.tile_pool(name="ypool", bufs=1) as ypool,
            tc.tile_pool(name="hpool", bufs=2) as hpool,
            tc.tile_pool(name="w1pool", bufs=3) as w1pool,
            tc.tile_pool(name="w2pool", bufs=3) as w2pool,
            tc.tile_pool(name="phpool", bufs=4, space="PSUM") as phpool,
            tc.tile_pool(name="pypool", bufs=4, space="PSUM") as pypool,
        ):
            b1t = cpool.tile([P, D_HID // P], f32)
            nc.sync.dma_start(b1t[:], b1c[:])
            # prefetch the first w1 tile so the first matmul group is not
            # stuck behind the 16 x-tile DMAs
            w1_pre = w1pool.tile([P, D_IN], bf16, tag="w1")
            nc.sync.dma_start(w1_pre[:], w1p[0])

            g0 = 0
            for g, tg in enumerate(groups):
                widths = _widths_for(tg)
                nt = -(-tg // P)
                xs = [xpool.tile([P, tgmax], bf16, tag=f"x{kt}",
                                 name=f"xs{kt}")
                      for kt in range(KT)]
                for kt in range(KT):
                    nc.sync.dma_start(
                        xs[kt][:, :tg], xT[kt * P:(kt + 1) * P, g0:g0 + tg])
                ys = [ypool.tile([P, D_OUT], f32, tag=f"y{t}", name=f"ys{t}")
                      for t in range(nt)]
                for b in range(NBLK):
                    hs = [hpool.tile([P, tgmax], bf16, tag=f"h{i}",
                                     name=f"hs{i}")
                          for i in range(HPB)]
                    # ---- matmul 1: h[hid, tok] = w1 @ x, gelu ----
                    for hb in range(HPB):
                        hid0 = b * HPB + hb
                        if g == 0 and b == 0 and hb == 0:
                            w1t = w1_pre
                        else:
                            w1t = w1pool.tile([P, D_IN], bf16, tag="w1")
                            nc.sync.dma_start(w1t[:], w1p[hid0])
                        tw0 = 0
                        for tw in widths:
                            ph = phpool.tile([P, 512], mybir.dt.float32,
                                             tag="ph")
                            for kt in range(KT):
                                nc.tensor.matmul(
                                    ph[:, :tw],
                                    w1t[:, kt * P:(kt + 1) * P],
                                    xs[kt][:, tw0:tw0 + tw],
                                    start=(kt == 0), stop=(kt == KT - 1))
                            nc.scalar.activation(
                                hs[hb][:, tw0:tw0 + tw], ph[:, :tw],
                                gelu, bias=b1t[:, hid0:hid0 + 1])
                            tw0 += tw
                    # ---- matmul 2: y[tok, dout] += h_blk @ w2_blk ----
                    for dc in range(NDC):
                        w2t = w2pool.tile([P, HPB * DOUTW], bf16, tag="w2")
                        nc.sync.dma_start(w2t[:], w2p[b, dc])
                        for t in range(nt):
                            pt = min(P, tg - t * P)
                            py = pypool.tile([P, DOUTW], mybir.dt.float32,
                                             tag="py")
                            for i in range(HPB):
                                nc.tensor.matmul(
                                    py[:pt, :],
                                    hs[i][:, t * P:t * P + pt],
                                    w2t[:, i * DOUTW:(i + 1) * DOUTW],
                                    start=(i == 0), stop=(i == HPB - 1))
                            dst = ys[t][:pt, dc * DOUTW:(dc + 1) * DOUTW]
                            if b == 0:
                                nc.vector.tensor_copy(dst, py[:pt, :])
                            else:
                                nc.vector.tensor_add(dst, dst, py[:pt, :])
                                if b == NBLK - 1:
                                    nc.sync.dma_start(
                                        y[g0 + t * P:g0 + t * P + pt,
                                          dc * DOUTW:(dc + 1) * DOUTW],
                                        dst)
                g0 += tg
    nc.compile()
    return nc


def kernel(x, gate_w, w1, b1, w2, b2):
    global LAST_EXEC_TIME_NS, LAST_RESULTS
    x = np.asarray(x, dtype=np.float32)
    gate_w = np.asarray(gate_w, dtype=np.float32)
    w1 = np.asarray(w1, dtype=np.float32)
    b1 = np.asarray(b1, dtype=np.float32)
    w2 = np.asarray(w2, dtype=np.float32)
    b2 = np.asarray(b2, dtype=np.float32)
    B = x.shape[0]

    # ---- host router (fp32, matches jax.lax.top_k tie-breaking) ----
    logits = x @ gate_w.T                                     # [B, E]
    order = np.argsort(-logits, axis=1, kind="stable")[:, :TOP_K]
    top_v = np.take_along_axis(logits, order, axis=1)
    mx = top_v.max(axis=1, keepdims=True)
    ex = np.exp(top_v - mx)
    coefs = ex / ex.sum(axis=1, keepdims=True)                # [B, 2]

    toks, cfs = [], []
    for e in range(NUM_EXPERTS):
        mask = order == e                                     # [B, 2]
        tok = np.nonzero(mask.any(axis=1))[0]
        first = mask[tok, 0]
        cf = np.where(first, coefs[tok, 0], coefs[tok, 1]).astype(np.float32)
        toks.append(tok)
        cfs.append(cf)

    max_c = max(len(t) for t in toks)
    groups = _groups_for(max_c)
    C = sum(groups)

    # ---- per-core inputs: tokens + packed weights of the owned expert ----
    in_maps = []
    for e in range(NUM_EXPERTS):
        tok = toks[e]
        xg = np.zeros((C, D_IN), np.float32)
        xg[:len(tok)] = x[tok]
        xT = xg.T.astype(_BF16)                               # [D_IN, C]

        w1e = w1[e].astype(_BF16)                             # [HID, D_IN]
        w1p = (w1e.reshape(D_HID // P, P, KT, P)
               .transpose(0, 3, 2, 1)
               .reshape(D_HID // P, P, D_IN))
        w1p = np.ascontiguousarray(w1p)

        w2e = w2[e].astype(_BF16)                             # [D_OUT, HID]
        w2p = (w2e.reshape(NDC, DOUTW, NBLK, HPB, P)
               .transpose(2, 0, 4, 3, 1)
               .reshape(NBLK, NDC, P, HPB * DOUTW))
        w2p = np.ascontiguousarray(w2p)

        b1c = np.ascontiguousarray(b1[e].reshape(D_HID // P, P).T)

        in_maps.append({"xT": xT, "w1p": w1p, "w2p": w2p, "b1c": b1c})

    nc = _nc_cache.get(groups)
    if nc is None:
        nc = _build_bass(groups)
        _nc_cache[groups] = nc

    from concourse.bass_utils import run_bass_kernel_spmd
    res = run_bass_kernel_spmd(nc, in_maps, core_ids=list(range(NUM_EXPERTS)))
    LAST_EXEC_TIME_NS = res.exec_time_ns
    LAST_RESULTS = res

    # ---- combine (unshard): weighted scatter-add; b2[e] folded in here ----
    out = np.zeros((B, D_OUT), np.float32)
    for e in range(NUM_EXPERTS):
        tok = toks[e]
        y_e = np.asarray(res.results[e]["y"])[:len(tok)]
        out[tok] += (y_e + b2[e][None, :]) * cfs[e][:, None]
    return out


# revision 10
# speedup vs baseline: 1.2285x; 1.0226x over previous
"""MoE layer (8 experts, top-2) on 8 Trainium2 NeuronCores.

Strategy: expert parallelism. The router (x @ gate_w.T -> top-2 -> softmax)
is computed on host in fp32 (0.03% of total FLOPs); tokens are then
sharded BY EXPERT: core e receives the tokens routed to expert e (padded
to a fixed capacity C) plus expert e's weights, pre-packed into
DMA-friendly tiled layouts. Each core computes the dense expert MLP
    y = gelu(x @ w1[e].T + b1[e]) @ w2[e].T
in bf16 (fp32 PSUM accumulation). The combine (scatter-add weighted by the
top-2 softmax coefficients, with b2[e] folded in per expert) happens on
host as the unshard step.

Device kernel layout (per core, SPMD identical program):
  xT  [D_IN, C]  bf16   tokens, transposed (partition dim = contraction)
  w1p [64, 128, 2048]   w1 tiles: w1p[h0, p, kt*128+h] = w1[e][h0*128+h, kt*128+p]
  w2p [8, 4, 128, 4096] w2 [hid128, dout128] tiles, grouped by (hid block,
                        dout quarter) so matmul-2 keeps w2 stationary and
                        streams token columns (token count stays exact)
  b1c [128, 64]         b1 per hid-tile column
  y   [D_OUT, C] f32    expert output, transposed (excl. b2/routing coef)

Inner loop: token groups (<=1152, exact capacity, last group ragged); hid
blocked by 1024 (h stays in SBUF as bf16); y accumulated in SBUF fp32 via
DVE adds across hid blocks, streamed out per 512-wide chunk of the last
block. Measured: 1.87 ms HW exec (8 cores, ~94.7% tensor-engine MFU),
rel err 3.4e-3 vs the fp32 reference.
"""

import numpy as np
import ml_dtypes

TOP_K = 2
NUM_EXPERTS = 8
D_IN, D_HID, D_OUT = 2048, 8192, 2048

P = 128
TG = 1152          # tokens per group (SBUF-resident)
TOKW = 384         # matmul-1 moving width; TG = 3 * TOKW
DOUTW = 512        # matmul-2 moving width; D_OUT = 4 * DOUTW
KT = D_IN // P     # 16 contraction tiles
NBLK = 8           # hid blocks of 1024
HPB = 8            # hid 128-tiles per block
NT = TG // P       # 9 token 128-tiles per group
NTW = TG // TOKW   # 3
NDC = D_OUT // DOUTW  # 4

_BF16 = ml_dtypes.bfloat16

_nc_cache: dict[int, object] = {}

LAST_EXEC_TIME_NS = None
LAST_RESULTS = None


def _groups_for(max_c: int) -> tuple[int, ...]:
    """Token-group sizes (<= TG) covering exactly max_c tokens."""
    c = max(max_c, 1)
    groups = []
    while c > TG:
        groups.append(TG)
        c -= TG
    groups.append(c)
    return tuple(groups)


def _widths_for(tg: int) -> list[int]:
    """Split a group into matmul-1 moving widths (<= 512), each starting at
    a 128-aligned token offset (only the last may be a non-multiple)."""
    if tg % 384 == 0 and tg % 512 != 0:
        return [384] * (tg // 384)
    ws = [512] * (tg // 512)
    if tg % 512:
        ws.append(tg % 512)
    return ws


def _build_bass(groups: tuple[int, ...]):
    from concourse import bacc
    import concourse.mybir as mybir
    import concourse.tile as tile

    bf16 = mybir.dt.bfloat16
    f32 = mybir.dt.float32
    C = sum(groups)
    tgmax = max(groups)

    nc = bacc.Bacc("TRN2", target_bir_lowering=False, debug=False,
                   num_devices=NUM_EXPERTS)
    xT = nc.declare_dram_parameter("xT", [D_IN, C], bf16, isOutput=False)
    w1p = nc.declare_dram_parameter("w1p", [D_HID // P, P, D_IN], bf16,
                                    isOutput=False)
    w2p = nc.declare_dram_parameter("w2p", [NBLK, NDC, P, HPB * NDC * P],
                                    bf16, isOutput=False)
    b1c = nc.declare_dram_parameter("b1c", [P, D_HID // P], f32, isOutput=False)
    y = nc.declare_dram_parameter("y", [D_OUT, C], f32, isOutput=True)

    gelu = mybir.ActivationFunctionType.Gelu

    with tile.TileContext(nc) as tc:
        with (
            tc.tile_pool(name="consts", bufs=1) as cpool,
            tc.tile_pool(name="xpool", bufs=1) as xpool,
            tc.tile_pool(name="ypool", bufs=1) as ypool,
            tc.tile_pool(name="hpool", bufs=2) as hpool,
            tc.tile_pool(name="w1pool", bufs=3) as w1pool,
            tc.tile_pool(name="w2pool", bufs=3) as w2pool,
            tc.tile_pool(name="phpool", bufs=4, space="PSUM") as phpool,
            tc.tile_pool(name="pypool", bufs=4, space="PSUM") as pypool,
        ):
            b1t = cpool.tile([P, D_HID // P], f32)
            nc.sync.dma_start(b1t[:], b1c[:])
            # prefetch the first w1 tile so the first matmul group is not
            # stuck behind the 16 x-tile DMAs
            w1_pre = w1pool.tile([P, D_IN], bf16, tag="w1")
            nc.sync.dma_start(w1_pre[:], w1p[0])

            g0 = 0
            for g, tg in enumerate(groups):
                widths = _widths_for(tg)
                nt = -(-tg // P)
                xs = [xpool.tile([P, tgmax], bf16, tag=f"x{kt}",
                                 name=f"xs{kt}")
                      for kt in range(KT)]
                for kt in range(KT):
                    nc.sync.dma_start(
                        xs[kt][:, :tg], xT[kt * P:(kt + 1) * P, g0:g0 + tg])
                ys = [ypool.tile([P, tgmax], f32, tag=f"y{t}", name=f"ys{t}")
                      for t in range(D_OUT // P)]
                for b in range(NBLK):
                    hs = [hpool.tile([P, tgmax], bf16, tag=f"h{i}",
                                     name=f"hs{i}")
                          for i in range(HPB)]
                    # ---- matmul 1: h[hid, tok] = w1 @ x, gelu ----
                    for hb in range(HPB):
                        hid0 = b * HPB + hb
                        if g == 0 and b == 0 and hb == 0:
                            w1t = w1_pre
                        else:
                            w1t = w1pool.tile([P, D_IN], bf16, tag="w1")
                            nc.sync.dma_start(w1t[:], w1p[hid0])
                        tw0 = 0
                        for tw in widths:
                            ph = phpool.tile([P, 512], mybir.dt.float32,
                                             tag="ph")
                            for kt in range(KT):
                                nc.tensor.matmul(
                                    ph[:, :tw],
                                    w1t[:, kt * P:(kt + 1) * P],
                                    xs[kt][:, tw0:tw0 + tw],
                                    start=(kt == 0), stop=(kt == KT - 1))
                            nc.scalar.activation(
                                hs[hb][:, tw0:tw0 + tw], ph[:, :tw],
                                gelu, bias=b1t[:, hid0:hid0 + 1])
                            tw0 += tw
                    # ---- matmul 2: yT[dout, tok] += w2_blk.T-tiles @ h ----
                    # stationary = w2 [hid128, dout128] tiles, moving = h
                    # token chunks; tokens are exact (no ceil-to-128 waste)
                    for q in range(NDC):
                        w2t = w2pool.tile([P, HPB * NDC * P], bf16, tag="w2")
                        nc.sync.dma_start(w2t[:], w2p[b, q])
                        for dtl in range(NDC):
                            dt = q * NDC + dtl
                            ch0 = 0
                            for cw in widths:
                                py = pypool.tile([P, DOUTW], mybir.dt.float32,
                                                 tag="py")
                                for i in range(HPB):
                                    nc.tensor.matmul(
                                        py[:, :cw],
                                        w2t[:, (i * NDC + dtl) * P:
                                            (i * NDC + dtl + 1) * P],
                                        hs[i][:, ch0:ch0 + cw],
                                        start=(i == 0), stop=(i == HPB - 1))
                                dst = ys[dt][:, ch0:ch0 + cw]
                                if b == 0:
                                    nc.vector.tensor_copy(dst, py[:, :cw])
                                else:
                                    nc.vector.tensor_add(dst, dst, py[:, :cw])
                                    if b == NBLK - 1:
                                        nc.sync.dma_start(
                                            y[dt * P:(dt + 1) * P,
                                              g0 + ch0:g0 + ch0 + cw],
                                            dst)
                                ch0 += cw
                g0 += tg
    nc.compile()
    return nc


def kernel(x, gate_w, w1, b1, w2, b2):
    global LAST_EXEC_TIME_NS, LAST_RESULTS
    x = np.asarray(x, dtype=np.float32)
    gate_w = np.asarray(gate_w, dtype=np.float32)
    w1 = np.asarray(w1, dtype=np.float32)
    b1 = np.asarray(b1, dtype=np.float32)
    w2 = np.asarray(w2, dtype=np.float32)
    b2 = np.asarray(b2, dtype=np.float32)
    B = x.shape[0]

    # ---- host router (fp32, matches jax.lax.top_k tie-breaking) ----
    logits = x @ gate_w.T                                     # [B, E]
    order = np.argsort(-logits, axis=1, kind="stable")[:, :TOP_K]
    top_v = np.take_along_axis(logits, order, axis=1)
    mx = top_v.max(axis=1, keepdims=True)
    ex = np.exp(top_v - mx)
    coefs = ex / ex.sum(axis=1, keepdims=True)                # [B, 2]

    toks, cfs = [], []
    for e in range(NUM_EXPERTS):
        mask = order == e                                     # [B, 2]
        tok = np.nonzero(mask.any(axis=1))[0]
        first = mask[tok, 0]
        cf = np.where(first, coefs[tok, 0], coefs[tok, 1]).astype(np.float32)
        toks.append(tok)
        cfs.append(cf)

    max_c = max(len(t) for t in toks)
    groups = _groups_for(max_c)
    C = sum(groups)

    # ---- per-core inputs: tokens + packed weights of the owned expert ----
    in_maps = []
    for e in range(NUM_EXPERTS):
        tok = toks[e]
        xg = np.zeros((C, D_IN), np.float32)
        xg[:len(tok)] = x[tok]
        xT = xg.T.astype(_BF16)                               # [D_IN, C]

        w1e = w1[e].astype(_BF16)                             # [HID, D_IN]
        w1p = (w1e.reshape(D_HID // P, P, KT, P)
               .transpose(0, 3, 2, 1)
               .reshape(D_HID // P, P, D_IN))
        w1p = np.ascontiguousarray(w1p)

        w2e = w2[e].astype(_BF16)                             # [D_OUT, HID]
        w2p = (w2e.reshape(NDC, NDC, P, NBLK, HPB, P)     # [q, dtl, d, b, i, p]
               .transpose(3, 0, 5, 4, 1, 2)               # [b, q, p, i, dtl, d]
               .reshape(NBLK, NDC, P, HPB * NDC * P))
        w2p = np.ascontiguousarray(w2p)

        b1c = np.ascontiguousarray(b1[e].reshape(D_HID // P, P).T)

        in_maps.append({"xT": xT, "w1p": w1p, "w2p": w2p, "b1c": b1c})

    nc = _nc_cache.get(groups)
    if nc is None:
        nc = _build_bass(groups)
        _nc_cache[groups] = nc

    from concourse.bass_utils import run_bass_kernel_spmd
    res = run_bass_kernel_spmd(nc, in_maps, core_ids=list(range(NUM_EXPERTS)))
    LAST_EXEC_TIME_NS = res.exec_time_ns
    LAST_RESULTS = res

    # ---- combine (unshard): weighted scatter-add; b2[e] folded in here ----
    out = np.zeros((B, D_OUT), np.float32)
    for e in range(NUM_EXPERTS):
        tok = toks[e]
        y_e = np.asarray(res.results[e]["y"]).T[:len(tok)]
        out[tok] += (y_e + b2[e][None, :]) * cfs[e][:, None]
    return out


# revision 11
# speedup vs baseline: 1.2317x; 1.0026x over previous
"""MoE layer (8 experts, top-2) on 8 Trainium2 NeuronCores.

Strategy: expert parallelism. The router (x @ gate_w.T -> top-2 -> softmax)
is computed on host in fp32 (0.03% of total FLOPs); tokens are then
sharded BY EXPERT: core e receives the tokens routed to expert e (padded
to a fixed capacity C) plus expert e's weights, pre-packed into
DMA-friendly tiled layouts. Each core computes the dense expert MLP
    y = gelu(x @ w1[e].T + b1[e]) @ w2[e].T
in bf16 (fp32 PSUM accumulation). The combine (scatter-add weighted by the
top-2 softmax coefficients, with b2[e] folded in per expert) happens on
host as the unshard step.

Device kernel layout (per core, SPMD identical program):
  xT  [D_IN, C]  bf16   tokens, transposed (partition dim = contraction)
  w1p [64, 128, 2048]   w1 tiles: w1p[h0, p, kt*128+h] = w1[e][h0*128+h, kt*128+p]
  w2p [8, 4, 128, 4096] w2 [hid128, dout128] tiles, grouped by (hid block,
                        dout quarter) so matmul-2 keeps w2 stationary and
                        streams token columns (token count stays exact)
  b1c [128, 64]         b1 per hid-tile column
  y   [D_OUT, C] f32    expert output, transposed (excl. b2/routing coef)

Inner loop: token groups (<=1152, exact capacity, last group ragged); hid
blocked by 1024 (h stays in SBUF as bf16); y accumulated in SBUF fp32 via
DVE adds across hid blocks, streamed out per 512-wide chunk of the last
block. Measured: 1.833 ms HW exec (8 cores, 96-97% tensor-engine MFU,
PE busy within ~1% of the bf16 streaming floor), rel err 3.4e-3 vs the
fp32 reference.
"""

import numpy as np
import ml_dtypes

TOP_K = 2
NUM_EXPERTS = 8
D_IN, D_HID, D_OUT = 2048, 8192, 2048

P = 128
TG = 1152          # tokens per group (SBUF-resident)
TOKW = 384         # matmul-1 moving width; TG = 3 * TOKW
DOUTW = 512        # matmul-2 moving width; D_OUT = 4 * DOUTW
KT = D_IN // P     # 16 contraction tiles
NBLK = 8           # hid blocks of 1024
HPB = 8            # hid 128-tiles per block
NT = TG // P       # 9 token 128-tiles per group
NTW = TG // TOKW   # 3
NDC = D_OUT // DOUTW  # 4

_BF16 = ml_dtypes.bfloat16

_nc_cache: dict[int, object] = {}

LAST_EXEC_TIME_NS = None
LAST_RESULTS = None


def _groups_for(max_c: int) -> tuple[int, ...]:
    """Token-group sizes (<= TG) covering exactly max_c tokens."""
    c = max(max_c, 1)
    groups = []
    while c > TG:
        groups.append(TG)
        c -= TG
    groups.append(c)
    return tuple(groups)


def _widths_for(tg: int) -> list[int]:
    """Split a group into matmul-1 moving widths (<= 512), each starting at
    a 128-aligned token offset (only the last may be a non-multiple)."""
    if tg % 384 == 0 and tg % 512 != 0:
        return [384] * (tg // 384)
    ws = [512] * (tg // 512)
    if tg % 512:
        ws.append(tg % 512)
    return ws


def _build_bass(groups: tuple[int, ...]):
    from concourse import bacc
    import concourse.mybir as mybir
    import concourse.tile as tile

    bf16 = mybir.dt.bfloat16
    f32 = mybir.dt.float32
    C = sum(groups)
    tgmax = max(groups)

    nc = bacc.Bacc("TRN2", target_bir_lowering=False, debug=False,
                   num_devices=NUM_EXPERTS)
    xT = nc.declare_dram_parameter("xT", [D_IN, C], bf16, isOutput=False)
    w1p = nc.declare_dram_parameter("w1p", [D_HID // P, P, D_IN], bf16,
                                    isOutput=False)
    w2p = nc.declare_dram_parameter("w2p", [NBLK, NDC, P, HPB * NDC * P],
                                    bf16, isOutput=False)
    b1c = nc.declare_dram_parameter("b1c", [P, D_HID // P], f32, isOutput=False)
    y = nc.declare_dram_parameter("y", [D_OUT, C], f32, isOutput=True)

    gelu = mybir.ActivationFunctionType.Gelu

    with tile.TileContext(nc) as tc:
        with (
            tc.tile_pool(name="consts", bufs=1) as cpool,
            tc.tile_pool(name="xpool", bufs=1) as xpool,
            tc.tile_pool(name="ypool", bufs=1) as ypool,
            tc.tile_pool(name="hpool", bufs=2) as hpool,
            tc.tile_pool(name="w1pool", bufs=3) as w1pool,
            tc.tile_pool(name="w2pool", bufs=3) as w2pool,
            tc.tile_pool(name="phpool", bufs=4, space="PSUM") as phpool,
            tc.tile_pool(name="pypool", bufs=4, space="PSUM") as pypool,
        ):
            # prefetch the first w1 tile so the first matmul group is not
            # stuck behind the 16 x-tile DMAs
            w1_pre = w1pool.tile([P, D_IN], bf16, tag="w1")
            nc.sync.dma_start(w1_pre[:], w1p[0])
            b1t = cpool.tile([P, D_HID // P], f32)
            nc.sync.dma_start(b1t[:], b1c[:])

            g0 = 0
            for g, tg in enumerate(groups):
                widths = _widths_for(tg)
                nt = -(-tg // P)
                xs = [xpool.tile([P, tgmax], bf16, tag=f"x{kt}",
                                 name=f"xs{kt}")
                      for kt in range(KT)]
                for kt in range(KT):
                    nc.sync.dma_start(
                        xs[kt][:, :tg], xT[kt * P:(kt + 1) * P, g0:g0 + tg])
                ys = [ypool.tile([P, tgmax], f32, tag=f"y{t}", name=f"ys{t}")
                      for t in range(D_OUT // P)]
                for b in range(NBLK):
                    hs = [hpool.tile([P, tgmax], bf16, tag=f"h{i}",
                                     name=f"hs{i}")
                          for i in range(HPB)]
                    # ---- matmul 1: h[hid, tok] = w1 @ x, gelu ----
                    for hb in range(HPB):
                        hid0 = b * HPB + hb
                        if g == 0 and b == 0 and hb == 0:
                            w1t = w1_pre
                        else:
                            w1t = w1pool.tile([P, D_IN], bf16, tag="w1")
                            nc.sync.dma_start(w1t[:], w1p[hid0])
                        tw0 = 0
                        for tw in widths:
                            ph = phpool.tile([P, 512], mybir.dt.float32,
                                             tag="ph")
                            for kt in range(KT):
                                nc.tensor.matmul(
                                    ph[:, :tw],
                                    w1t[:, kt * P:(kt + 1) * P],
                                    xs[kt][:, tw0:tw0 + tw],
                                    start=(kt == 0), stop=(kt == KT - 1))
                            nc.scalar.activation(
                                hs[hb][:, tw0:tw0 + tw], ph[:, :tw],
                                gelu, bias=b1t[:, hid0:hid0 + 1])
                            tw0 += tw
                    # ---- matmul 2: yT[dout, tok] += w2_blk.T-tiles @ h ----
                    # stationary = w2 [hid128, dout128] tiles, moving = h
                    # token chunks; tokens are exact (no ceil-to-128 waste)
                    for q in range(NDC):
                        w2t = w2pool.tile([P, HPB * NDC * P], bf16, tag="w2")
                        nc.sync.dma_start(w2t[:], w2p[b, q])
                        for dtl in range(NDC):
                            dt = q * NDC + dtl
                            ch0 = 0
                            for cw in widths:
                                py = pypool.tile([P, DOUTW], mybir.dt.float32,
                                                 tag="py")
                                for i in range(HPB):
                                    nc.tensor.matmul(
                                        py[:, :cw],
                                        w2t[:, (i * NDC + dtl) * P:
                                            (i * NDC + dtl + 1) * P],
                                        hs[i][:, ch0:ch0 + cw],
                                        start=(i == 0), stop=(i == HPB - 1))
                                dst = ys[dt][:, ch0:ch0 + cw]
                                if b == 0:
                                    nc.vector.tensor_copy(dst, py[:, :cw])
                                else:
                                    nc.vector.tensor_add(dst, dst, py[:, :cw])
                                    if b == NBLK - 1:
                                        nc.sync.dma_start(
                                            y[dt * P:(dt + 1) * P,
                                              g0 + ch0:g0 + ch0 + cw],
                                            dst)
                                ch0 += cw
                g0 += tg
    nc.compile()
    return nc


def kernel(x, gate_w, w1, b1, w2, b2):
    global LAST_EXEC_TIME_NS, LAST_RESULTS
    x = np.asarray(x, dtype=np.float32)
    gate_w = np.asarray(gate_w, dtype=np.float32)
    w1 = np.asarray(w1, dtype=np.float32)
    b1 = np.asarray(b1, dtype=np.float32)
    w2 = np.asarray(w2, dtype=np.float32)
    b2 = np.asarray(b2, dtype=np.float32)
    B = x.shape[0]

    # ---- host router (fp32, matches jax.lax.top_k tie-breaking) ----
    logits = x @ gate_w.T                                     # [B, E]
    order = np.argsort(-logits, axis=1, kind="stable")[:, :TOP_K]
    top_v = np.take_along_axis(logits, order, axis=1)
    mx = top_v.max(axis=1, keepdims=True)
    ex = np.exp(top_v - mx)
    coefs = ex / ex.sum(axis=1, keepdims=True)                # [B, 2]

    toks, cfs = [], []
    for e in range(NUM_EXPERTS):
        mask = order == e                                     # [B, 2]
        tok = np.nonzero(mask.any(axis=1))[0]
        first = mask[tok, 0]
        cf = np.where(first, coefs[tok, 0], coefs[tok, 1]).astype(np.float32)
        toks.append(tok)
        cfs.append(cf)

    max_c = max(len(t) for t in toks)
    groups = _groups_for(max_c)
    C = sum(groups)

    # ---- per-core inputs: tokens + packed weights of the owned expert ----
    in_maps = []
    for e in range(NUM_EXPERTS):
        tok = toks[e]
        xg = np.zeros((C, D_IN), np.float32)
        xg[:len(tok)] = x[tok]
        xT = xg.T.astype(_BF16)                               # [D_IN, C]

        w1e = w1[e].astype(_BF16)                             # [HID, D_IN]
        w1p = (w1e.reshape(D_HID // P, P, KT, P)
               .transpose(0, 3, 2, 1)
               .reshape(D_HID // P, P, D_IN))
        w1p = np.ascontiguousarray(w1p)

        w2e = w2[e].astype(_BF16)                             # [D_OUT, HID]
        w2p = (w2e.reshape(NDC, NDC, P, NBLK, HPB, P)     # [q, dtl, d, b, i, p]
               .transpose(3, 0, 5, 4, 1, 2)               # [b, q, p, i, dtl, d]
               .reshape(NBLK, NDC, P, HPB * NDC * P))
        w2p = np.ascontiguousarray(w2p)

        b1c = np.ascontiguousarray(b1[e].reshape(D_HID // P, P).T)

        in_maps.append({"xT": xT, "w1p": w1p, "w2p": w2p, "b1c": b1c})

    nc = _nc_cache.get(groups)
    if nc is None:
        nc = _build_bass(groups)
        _nc_cache[groups] = nc

    from concourse.bass_utils import run_bass_kernel_spmd
    res = run_bass_kernel_spmd(nc, in_maps, core_ids=list(range(NUM_EXPERTS)))
    LAST_EXEC_TIME_NS = res.exec_time_ns
    LAST_RESULTS = res

    # ---- combine (unshard): weighted scatter-add; b2[e] folded in here ----
    out = np.zeros((B, D_OUT), np.float32)
    for e in range(NUM_EXPERTS):
        tok = toks[e]
        y_e = np.asarray(res.results[e]["y"]).T[:len(tok)]
        out[tok] += (y_e + b2[e][None, :]) * cfs[e][:, None]
    return out
